# revision 1
# baseline (speedup 1.0000x reference)
"""Trainium2 Bass kernel for nn_EdgeUpdater (GNN message passing edge update).

Computes, for E=2M edges with node tables of 100k rows (C=32):
    v = relu(relu(var_f @ Wv1.T + bv1) @ Wv2.T + bv2)
    c = relu(relu(con_f @ Wc1.T + bc1) @ Wc2.T + bc2)
    x = concat([combined_edge_f, v[i0], c[i1]], axis=1)      # [E, 3C]
    out = relu(x @ We1.T + be1) @ We2.T + be2                # [E, C]

Strategy (8 cores, edge-sharded, node tables replicated per core):
  Algebraic split of We1 = [A | B | Cc] over the concat:
    h  = relu(e @ A.T + Pv[i0] + Pc[i1]),  out = h @ We2.T + be2
    Pv = v @ B.T + be1   (be1 folded once),  Pc = c @ Cc.T
  Per core: compute Pv/Pc tables (fp16, DRAM), then stream edge shard:
  row-gather Pv[i0]+Pc[i1] via SWDGE indirect DMA (the second gather
  accumulates with accum_op=add), fp16 matmuls with 4x block-diagonal
  (kron(I4, W.T)) stationaries over a group-cyclic layout, DVE 32x32
  stream-transposes for row-major <-> channel-major conversion.
"""

import numpy as np

import concourse.bass as bass
import concourse.mybir as mybir
import concourse.tile as tile
from concourse.bass import IndirectOffsetOnAxis
from concourse.bass_utils import run_bass_kernel_spmd

C = 32
P = 128
NB = 64  # rows per partition per macro tile
MACRO = P * NB  # 8192 rows per macro tile
BANK = 512  # psum bank free dim (fp32)
NBANKS = (NB * C) // BANK  # 4
N_CORES = 8

F32 = mybir.dt.float32
F16 = mybir.dt.float16
I32 = mybir.dt.int32

RELU = mybir.ActivationFunctionType.Relu
IDENT = mybir.ActivationFunctionType.Identity
ADD = mybir.AluOpType.add


def _split_multi_waits(nc: bass.Bass, max_waits: int = 1):
    """The walrus in this container rejects instructions carrying more than
    one sync wait ("Too many sync wait commands", CoreV2/V3 setupSyncWait).
    Hoist extra waits onto single-wait NOPs on the same engine, inserted
    immediately before the instruction (same per-engine program order, so
    semantics are unchanged)."""
    for fn in nc.m.functions:
        for bb in fn.blocks:
            insts = list(bb.instructions)
            out = []
            for ins in insts:
                si = ins.sync_info
                if (si is not None and si.on_wait
                        and len(si.on_wait) > max_waits
                        and ins.engine is not None):
                    waits = list(si.on_wait)
                    eng = nc.engines[ins.engine]
                    for w in waits[:-max_waits]:
                        nop = eng.nop()
                        cur = nc.cur_bb.bb
                        assert cur.instructions[-1] is nop.ins
                        cur.instructions.pop()
                        nop.ins.sync_info = mybir.SyncInfo(
                            on_wait=[w], on_update=[])
                        out.append(nop.ins)
                    si.on_wait = waits[-max_waits:]
                out.append(ins)
            bb.instructions.clear()
            for ins in out:
                bb.instructions.append(ins)


def build_nc(me: int, mn: int) -> bass.Bass:
    """Build the per-core Bass module.

    me: number of 8192-edge macro tiles in the edge shard
    mn: number of 8192-row macro tiles per node table
    """
    nc = bass.Bass()

    ef = nc.declare_dram_parameter("ef", [me, P, NB, C], F32, isOutput=False)
    i0 = nc.declare_dram_parameter("i0", [me, P, NB], I32, isOutput=False)
    i1 = nc.declare_dram_parameter("i1", [me, P, NB], I32, isOutput=False)
    vf = nc.declare_dram_parameter("vf", [mn, P, NB, C], F32, isOutput=False)
    cf = nc.declare_dram_parameter("cf", [mn, P, NB, C], F32, isOutput=False)

    wnames = ["a_st", "w2_st", "ident", "wv1_st", "wv2_st", "pv_st",
              "wc1_st", "wc2_st", "pc_st"]
    wparams = {n: nc.declare_dram_parameter(n, [P, P], F16, isOutput=False)
               for n in wnames}
    bnames = ["bv1_t", "bv2_t", "bc1_t", "bc2_t", "be1_t", "be2_t"]
    bparams = {n: nc.declare_dram_parameter(n, [P, 1], F32, isOutput=False)
               for n in bnames}

    out = nc.declare_dram_parameter("out", [me, P, NB, C], F32, isOutput=True)

    n_nodes_pad = mn * MACRO
    pv_tab = nc.dram_tensor("pv_tab", [n_nodes_pad, C], F16)
    pc_tab = nc.dram_tensor("pc_tab", [n_nodes_pad, C], F16)

    with tile.TileContext(nc) as tc:
        with tc.tile_pool(name="const", bufs=1) as cpool:
            W = {}
            for n in wnames:
                t = cpool.tile([P, P], F16, tag=n)
                nc.sync.dma_start(t[:], wparams[n][:])
                W[n] = t
            B = {}
            for n in bnames:
                t = cpool.tile([P, 1], F32, tag=n)
                nc.sync.dma_start(t[:], bparams[n][:])
                B[n] = t

            # One pool scope for both phases so the scheduler can overlap
            # edge v-gathers (Pool, depend only on pv_tab) with the Pc node
            # macros (PE/ACT/DVE). Emission order: Pv -> Pc -> edges.
            with tc.tile_pool(name="sb", bufs=2) as pool, \
                 tc.tile_pool(name="psum", bufs=2, space="PSUM") as psum:

                def node_macro(src, tab_view, w1, w2, w3, b1, b2, b3, mi):
                    L = pool.tile([P, NB * C], F16, tag="nL")
                    nc.gpsimd.dma_start(L[:], src[mi])  # f32->f16 (Pool idle here)
                    X = pool.tile([P, NB * C], F16, tag="nX")
                    nc.vector.transpose(X[:], L[:])
                    pcm = pool.tile([P, NB * C], F16, tag="npcm")
                    for q in range(NBANKS):
                        sl = slice(q * BANK, (q + 1) * BANK)
                        ps1 = psum.tile([P, BANK], F32, tag="nps1")
                        nc.tensor.matmul(ps1[:], lhsT=w1[:], rhs=X[:, sl],
                                         start=True, stop=True)
                        l1 = pool.tile([P, BANK], F16, tag="nl1")
                        nc.scalar.activation(l1[:], ps1[:], RELU, bias=b1[:])
                        ps2 = psum.tile([P, BANK], F32, tag="nps2")
                        nc.tensor.matmul(ps2[:], lhsT=w2[:], rhs=l1[:],
                                         start=True, stop=True)
                        l2 = pool.tile([P, BANK], F16, tag="nl2")
                        nc.scalar.activation(l2[:], ps2[:], RELU, bias=b2[:])
                        ps3 = psum.tile([P, BANK], F32, tag="nps1")
                        nc.tensor.matmul(ps3[:], lhsT=w3[:], rhs=l2[:],
                                         start=True, stop=True)
                        if b3 is not None:
                            nc.scalar.activation(pcm[:, sl], ps3[:], IDENT,
                                                 bias=b3[:])
                        else:
                            nc.scalar.activation(pcm[:, sl], ps3[:], IDENT)
                    pem = pool.tile([P, NB * C], F16, tag="npem")
                    nc.vector.transpose(pem[:], pcm[:])
                    nc.sync.dma_start(tab_view[mi], pem[:])

                pv_view = pv_tab[:].rearrange("(m p b) c -> m p b c", p=P, b=NB)
                pc_view = pc_tab[:].rearrange("(m p b) c -> m p b c", p=P, b=NB)
                for mi in range(mn):
                    node_macro(vf, pv_view, W["wv1_st"], W["wv2_st"], W["pv_st"],
                               B["bv1_t"], B["bv2_t"], B["be1_t"], mi)
                for mi in range(mn):
                    node_macro(cf, pc_view, W["wc1_st"], W["wc2_st"], W["pc_st"],
                               B["bc1_t"], B["bc2_t"], None, mi)

                for mi in range(me):
                    Lf = pool.tile([P, NB * C], F32, tag="eLf")
                    nc.sync.dma_start(Lf[:], ef[mi])
                    L = pool.tile([P, NB * C], F16, tag="eL")
                    nc.vector.tensor_copy(L[:], Lf[:])  # f32 -> f16
                    X = pool.tile([P, NB * C], F16, tag="eX")
                    nc.vector.transpose(X[:], L[:])

                    t0 = pool.tile([P, NB], I32, tag="ei0")
                    nc.sync.dma_start(t0[:], i0[mi])
                    t1 = pool.tile([P, NB], I32, tag="ei1")
                    nc.sync.dma_start(t1[:], i1[mi])

                    # HW indirect DMA gathers one row per partition per
                    # instruction (offset = first index of each partition row,
                    # span = out free extent), so gather row-column by
                    # row-column.
                    Sv = pool.tile([P, NB * C], F16, tag="eSv")
                    Sc = pool.tile([P, NB * C], F16, tag="eSc")
                    for s in range(NB):
                        nc.gpsimd.indirect_dma_start(
                            out=Sv[:, s * C:(s + 1) * C], out_offset=None,
                            in_=pv_tab[:],
                            in_offset=IndirectOffsetOnAxis(
                                ap=t0[:, s:s + 1], axis=0))
                    for s in range(NB):
                        nc.gpsimd.indirect_dma_start(
                            out=Sc[:, s * C:(s + 1) * C], out_offset=None,
                            in_=pc_tab[:],
                            in_offset=IndirectOffsetOnAxis(
                                ap=t1[:, s:s + 1], axis=0))
                    S = pool.tile([P, NB * C], F16, tag="eS")
                    nc.vector.tensor_add(S[:], Sv[:], Sc[:])
                    ST = pool.tile([P, NB * C], F16, tag="eST")
                    nc.vector.transpose(ST[:], S[:])

                    ocm = pool.tile([P, NB * C], F32, tag="eocm")
                    for q in range(NBANKS):
                        sl = slice(q * BANK, (q + 1) * BANK)
                        ps1 = psum.tile([P, BANK], F32, tag="eps1")
                        nc.tensor.matmul(ps1[:], lhsT=W["a_st"][:], rhs=X[:, sl],
                                         start=True, stop=False)
                        nc.tensor.matmul(ps1[:], lhsT=W["ident"][:], rhs=ST[:, sl],
                                         start=False, stop=True)
                        h = pool.tile([P, BANK], F16, tag="eh")
                        nc.scalar.activation(h[:], ps1[:], RELU)
                        ps2 = psum.tile([P, BANK], F32, tag="eps2")
                        nc.tensor.matmul(ps2[:], lhsT=W["w2_st"][:], rhs=h[:],
                                         start=True, stop=True)
                        nc.scalar.activation(ocm[:, sl], ps2[:], IDENT,
                                             bias=B["be2_t"][:])
                    oem = pool.tile([P, NB * C], F32, tag="eoem")
                    nc.vector.transpose(oem[:], ocm[:])
                    nc.sync.dma_start(out[mi], oem[:])

    _split_multi_waits(nc)
    return nc


def _kron4(w: np.ndarray) -> np.ndarray:
    return np.kron(np.eye(4, dtype=np.float32), w).astype(np.float16)


def _bias_t(b: np.ndarray) -> np.ndarray:
    return np.tile(np.asarray(b, np.float32), 4)[:, None].astype(np.float32)


def make_weight_inputs(Wv1, bv1, Wv2, bv2, Wc1, bc1, Wc2, bc2, We1, be1, We2, be2):
    We1 = np.asarray(We1, np.float32)
    return {
        "a_st": _kron4(np.asarray(We1[:, :C]).T.astype(np.float32)),
        "w2_st": _kron4(np.asarray(We2, np.float32).T),
        "ident": np.eye(P, dtype=np.float16),
        "wv1_st": _kron4(np.asarray(Wv1, np.float32).T),
        "wv2_st": _kron4(np.asarray(Wv2, np.float32).T),
        "pv_st": _kron4(We1[:, C:2 * C].T),
        "wc1_st": _kron4(np.asarray(Wc1, np.float32).T),
        "wc2_st": _kron4(np.asarray(Wc2, np.float32).T),
        "pc_st": _kron4(We1[:, 2 * C:3 * C].T),
        "bv1_t": _bias_t(bv1),
        "bv2_t": _bias_t(bv2),
        "bc1_t": _bias_t(bc1),
        "bc2_t": _bias_t(bc2),
        "be1_t": _bias_t(be1),
        "be2_t": _bias_t(be2),
    }


def _pad_nodes(x: np.ndarray, mn: int) -> np.ndarray:
    n_pad = mn * MACRO
    xp = np.zeros((n_pad, C), np.float32)
    xp[: x.shape[0]] = x
    return xp.reshape(mn, P, NB, C)


_NC_CACHE: dict = {}


def _get_nc(me: int, mn: int) -> bass.Bass:
    key = (me, mn)
    if key not in _NC_CACHE:
        _NC_CACHE[key] = build_nc(me, mn)
    return _NC_CACHE[key]


def kernel(var_f, con_f, combined_edge_f, edge_index_var_con,
           Wv1, bv1, Wv2, bv2, Wc1, bc1, Wc2, bc2, We1, be1, We2, be2,
           _trace=False, _tmpdir=None):
    var_f = np.asarray(var_f, np.float32)
    con_f = np.asarray(con_f, np.float32)
    combined_edge_f = np.asarray(combined_edge_f, np.float32)
    eidx = np.asarray(edge_index_var_con)

    E = combined_edge_f.shape[0]
    per = -(-E // N_CORES)
    me = -(-per // MACRO)
    e_pad = me * MACRO
    mn = -(-max(var_f.shape[0], con_f.shape[0]) // MACRO)

    base = make_weight_inputs(Wv1, bv1, Wv2, bv2, Wc1, bc1, Wc2, bc2,
                              We1, be1, We2, be2)
    base["vf"] = _pad_nodes(var_f, mn)
    base["cf"] = _pad_nodes(con_f, mn)

    i0_full = eidx[0].astype(np.int32)
    i1_full = eidx[1].astype(np.int32)

    in_maps = []
    shard_lens = []
    for k in range(N_CORES):
        lo = k * per
        hi = min(lo + per, E)
        n = hi - lo
        shard_lens.append(n)
        ef_k = np.zeros((e_pad, C), np.float32)
        ef_k[:n] = combined_edge_f[lo:hi]
        i0_k = np.zeros((e_pad,), np.int32)
        i0_k[:n] = i0_full[lo:hi]
        i1_k = np.zeros((e_pad,), np.int32)
        i1_k[:n] = i1_full[lo:hi]
        m = dict(base)
        m["ef"] = ef_k.reshape(me, P, NB, C)
        m["i0"] = i0_k.reshape(me, P, NB)
        m["i1"] = i1_k.reshape(me, P, NB)
        in_maps.append(m)

    nc = _get_nc(me, mn)
    res = run_bass_kernel_spmd(nc, in_maps, list(range(N_CORES)),
                               trace=_trace, tmpdir=_tmpdir)

    outs = []
    for k in range(N_CORES):
        o = np.asarray(res.results[k]["out"]).reshape(e_pad, C)
        outs.append(o[: shard_lens[k]])
    full = np.concatenate(outs, axis=0)
    if _trace:
        return full, res
    return full



# revision 42
# speedup vs baseline: 6.1138x; 6.1138x over previous
"""Trainium2 Bass kernel for nn_EdgeUpdater (GNN message passing edge update).

Computes, for E=2M edges with node tables of 100k rows (C=32):
    v = relu(relu(var_f @ Wv1.T + bv1) @ Wv2.T + bv2)
    c = relu(relu(con_f @ Wc1.T + bc1) @ Wc2.T + bc2)
    x = concat([combined_edge_f, v[i0], c[i1]], axis=1)      # [E, 3C]
    out = relu(x @ We1.T + be1) @ We2.T + be2                # [E, C]

Strategy (8 cores, edge-sharded, node tables replicated per core):
  Algebraic split of We1 = [A | B | Cc] over the concat:
    h  = relu(A@e + Pv[i0] + Pc[i1] + be1),  out = We2@h + be2
    Pv = v @ B.T,  Pc = c @ Cc.T   (be1 applied via the relu ACT bias)
  Per core: compute Pv/Pc tables as f32 rows padded to 64 f32 (256B) in
  DRAM, then stream edge tiles of 8192:
  - The per-edge random gather uses SWDGE dma_gather (InstDMAGatherAnt,
    'mlp' GPSIMD library): 1024 rows per instruction, int16 indices.
    To satisfy the int16 range, nodes are remapped host-side into 4 table
    chunks of 32768 rows, and edges are class-sorted host-side by
    (chunk(i0), chunk(i1)) into 16 fixed-size slot regions per core, so
    every 1024-slot gather window reads one statically-known chunk pair
    (the module stays SPMD-uniform across cores).
  - Edge features arrive host-pre-transposed into the 4x block-diagonal
    (kron(I4, W)) channel-major layout; outputs are stored channel-major
    and un-transposed host-side. Gathered rows are extract-added on DVE
    and 32x32 stream-transposed into the same layout.
"""

import numpy as np

import concourse.bass as bass
import concourse.mybir as mybir
import concourse.tile as tile
from concourse import library_config
from concourse.bass_utils import run_bass_kernel_spmd

C = 32
P = 128
NB = 64                 # j-slots per edge tile
TILE = P * NB           # 8192 edges per tile
WIN = 1024              # idx slots per dma_gather instruction
NWIN = TILE // WIN      # 8 windows per tile
NCHUNK = 4
CHUNK = 32768           # table rows per chunk
TAB_ROWS = NCHUNK * CHUNK
ROW_W = 64              # f32 per table row (256B; payload in cols 0:32)
NODE_CAP = 25088        # real node rows per chunk (3*8192 + 512)
NCLS = NCHUNK * NCHUNK  # 16 (chunk_v, chunk_c) classes
CLS_SLOTS0 = 16384      # default slots per class per core
N_CORES = 8
PART_NB = 4             # j-slots of the partial node macro (512 rows)
# Edge classes (cv, cc) ordered by readiness under the chunk-interleaved
# node schedule pv0,pc0,pv1,pc1,...: class usable once pv[cv] and pc[cc]
# are both written, i.e. after phase max(2*cv+1, 2*cc+2).
CLASS_ORDER = sorted(range(NCLS), key=lambda k: (
    max(2 * (k // NCHUNK) + 1, 2 * (k % NCHUNK) + 2), k))
CLASS_RANK = [CLASS_ORDER.index(k) for k in range(NCLS)]

F32 = mybir.dt.float32
F16 = mybir.dt.float16
I16 = mybir.dt.int16

RELU = mybir.ActivationFunctionType.Relu
IDENT = mybir.ActivationFunctionType.Identity


def _split_multi_waits(nc: bass.Bass, max_waits: int = 1):
    """The walrus in this container rejects instructions carrying more than
    one sync wait ("Too many sync wait commands", CoreV2/V3 setupSyncWait).
    Hoist extra waits onto single-wait NOPs on the same engine, inserted
    immediately before the instruction (same per-engine program order, so
    semantics are unchanged)."""
    for fn in nc.m.functions:
        for bb in fn.blocks:
            insts = list(bb.instructions)
            out = []
            for ins in insts:
                si = ins.sync_info
                if (si is not None and si.on_wait
                        and len(si.on_wait) > max_waits
                        and ins.engine is not None):
                    waits = list(si.on_wait)
                    eng = nc.engines[ins.engine]
                    for w in waits[:-max_waits]:
                        nop = eng.nop()
                        cur = nc.cur_bb.bb
                        assert cur.instructions[-1] is nop.ins
                        cur.instructions.pop()
                        nop.ins.sync_info = mybir.SyncInfo(
                            on_wait=[w], on_update=[])
                        out.append(nop.ins)
                    si.on_wait = waits[-max_waits:]
                out.append(ins)
            bb.instructions.clear()
            for ins in out:
                bb.instructions.append(ins)


def build_nc(me: int, cls_slots: int) -> bass.Bass:
    """Per-core module. me = edge tiles; cls_slots = slots per class
    (multiple of 1024; me*TILE == NCLS*cls_slots)."""
    assert me * TILE == NCLS * cls_slots
    wins_per_cls = cls_slots // WIN

    nc = bass.Bass()

    xcm = nc.declare_dram_parameter("xcm", [me, P, NB * C], F16, isOutput=False)
    idx = nc.declare_dram_parameter("idx", [me, P, TILE // 8], I16,
                                    isOutput=False)
    vfull = nc.declare_dram_parameter("vfull", [12, P, NB * C], F16,
                                      isOutput=False)
    vpart = nc.declare_dram_parameter("vpart", [NCHUNK, P, PART_NB * C], F16,
                                      isOutput=False)
    cfull = nc.declare_dram_parameter("cfull", [12, P, NB * C], F16,
                                      isOutput=False)
    cpart = nc.declare_dram_parameter("cpart", [NCHUNK, P, PART_NB * C], F16,
                                      isOutput=False)

    wnames = ["a_st", "ident", "w2_st", "wv1_st", "wv2_st", "pv_st",
              "wc1_st", "wc2_st", "pc_st"]
    wparams = {n: nc.declare_dram_parameter(n, [P, P], F16, isOutput=False)
               for n in wnames}
    bnames = ["bv1_t", "bv2_t", "bc1_t", "bc2_t", "be1_t", "be2_t"]
    bparams = {n: nc.declare_dram_parameter(n, [P, 1], F32, isOutput=False)
               for n in bnames}

    out = nc.declare_dram_parameter("out", [me, P, NB * C], F16, isOutput=True)

    pv_tabs = [nc.dram_tensor(f"pv_tab{c}", [CHUNK, ROW_W], F32)
               for c in range(NCHUNK)]
    pc_tabs = [nc.dram_tensor(f"pc_tab{c}", [CHUNK, ROW_W], F32)
               for c in range(NCHUNK)]

    with tile.TileContext(nc) as tc:
        nc.gpsimd.load_library(library_config.mlp)
        win_reg = nc.gpsimd.to_reg(WIN)
        # Preload an activation-function table containing both Relu and
        # Identity so no activation pays an implicit ACT_TABLE_LOAD.
        nc.scalar.add_instruction(mybir.InstLoadActFuncSet(
            name=nc.get_next_instruction_name(), ins=[], outs=[],
            act_func_set_id=0))
        with tc.tile_pool(name="const", bufs=1) as cpool:
            W = {}
            for n in wnames:
                t = cpool.tile([P, P], F16, tag=n)
                nc.sync.dma_start(t[:], wparams[n][:])
                W[n] = t
            B = {}
            for n in bnames:
                t = cpool.tile([P, 1], F32, tag=n)
                nc.sync.dma_start(t[:], bparams[n][:])
                B[n] = t
            with tc.tile_pool(name="sb", bufs=2) as pool, \
                 tc.tile_pool(name="psum", bufs=2, space="PSUM") as psum:

                def node_macro(src, src_i, tab, r0, nb, w1, w2, w3, b1, b2,
                               pool_relu1=False):
                    """MLP a macro of 128*nb node rows; write f32 rows
                    [r0 + p*64 + j] cols 0:32 of tab."""
                    fr = nb * C
                    X = pool.tile([P, NB * C], F16, tag="nX")
                    nc.sync.dma_start(X[:, :fr], src[src_i])
                    rm = pool.tile([P, NB * C], F32, tag="nrm")
                    nh = max(1, fr // WIN)
                    for h in range(nh):
                        w = min(WIN, fr)
                        sl = slice(h * WIN, h * WIN + w)
                        pA = psum.tile([P, WIN], F32, tag="psA")
                        for q0 in range(0, w, 512):
                            qw = min(512, w - q0)
                            nc.tensor.matmul(pA[:, q0:q0 + qw],
                                             lhsT=w1[:],
                                             rhs=X[:, h * WIN + q0:
                                                  h * WIN + q0 + qw],
                                             start=True, stop=True)
                        l1 = pool.tile([P, WIN], F16, tag="nl1")
                        if pool_relu1:
                            nc.gpsimd.tensor_scalar(
                                l1[:, :w], pA[:, :w], b1[:], 0.0,
                                mybir.AluOpType.add, mybir.AluOpType.max)
                        else:
                            nc.scalar.activation(l1[:, :w], pA[:, :w], RELU,
                                                 bias=b1[:])
                        pB = psum.tile([P, WIN], F32, tag="psB")
                        for q0 in range(0, w, 512):
                            qw = min(512, w - q0)
                            nc.tensor.matmul(pB[:, q0:q0 + qw],
                                             lhsT=w2[:],
                                             rhs=l1[:, q0:q0 + qw],
                                             start=True, stop=True)
                        l2 = pool.tile([P, WIN], F16, tag="nl2")
                        if pool_relu1:
                            nc.gpsimd.tensor_scalar(
                                l2[:, :w], pB[:, :w], b2[:], 0.0,
                                mybir.AluOpType.add, mybir.AluOpType.max)
                        else:
                            nc.scalar.activation(l2[:, :w], pB[:, :w], RELU,
                                                 bias=b2[:])
                        pC = psum.tile([P, WIN], F32, tag="psA")
                        for q0 in range(0, w, 512):
                            qw = min(512, w - q0)
                            nc.tensor.matmul(pC[:, q0:q0 + qw],
                                             lhsT=w3[:],
                                             rhs=l2[:, q0:q0 + qw],
                                             start=True, stop=True)
                        nc.vector.transpose(rm[:, sl], pC[:, :w])
                    # rm[p, j*C + c] = table value for row r0 + p*nb + j
                    rows = tab[r0:r0 + P * nb]
                    tv = rows.rearrange("(p j) w -> p j w", p=P)
                    nc.sync.dma_start(tv[:, :, 0:C],
                                  rm[:, :fr].rearrange("p (j c) -> p j c", c=C))

                def node_chunk(src_full, src_part, tab, c, w1, w2, w3,
                               b1, b2, pool_relu1=False):
                    for m in range(3):
                        node_macro(src_full, c * 3 + m, tab, m * TILE, NB,
                                   w1, w2, w3, b1, b2, pool_relu1)
                    node_macro(src_part, c, tab, 3 * TILE, PART_NB,
                               w1, w2, w3, b1, b2, pool_relu1)

                def edge_tile(mi):
                    Xt = pool.tile([P, NB * C], F16, tag="eX", bufs=3)
                    nc.sync.dma_start(Xt[:], xcm[mi])
                    tix = pool.tile([P, TILE // 8], I16, tag="eix", bufs=4)
                    nc.sync.dma_start(tix[:], idx[mi])

                    Gv = pool.tile([P, NB * ROW_W], F32, tag="eGv", bufs=3)
                    Gc = pool.tile([P, NB * ROW_W], F32, tag="eGc", bufs=3)
                    gvv = Gv[:].rearrange("p (n e) -> p n e", e=ROW_W)
                    gcv = Gc[:].rearrange("p (n e) -> p n e", e=ROW_W)
                    jw = WIN // P  # 8 j-slots per window
                    for w in range(NWIN):
                        k = CLASS_ORDER[(mi * NWIN + w) // wins_per_cls]
                        nc.gpsimd.dma_gather(
                            out_ap=gvv[:, w * jw:(w + 1) * jw],
                            in_ap=pv_tabs[k // NCHUNK][:],
                            idxs_ap=tix[:, w * (WIN // 16):(w + 1) * (WIN // 16)],
                            num_idxs=WIN, num_idxs_reg=win_reg,
                            elem_size=ROW_W, queue_num=0)
                        nc.gpsimd.dma_gather(
                            out_ap=gcv[:, w * jw:(w + 1) * jw],
                            in_ap=pc_tabs[k % NCHUNK][:],
                            idxs_ap=tix[:, TILE // 16 + w * (WIN // 16):
                                         TILE // 16 + (w + 1) * (WIN // 16)],
                            num_idxs=WIN, num_idxs_reg=win_reg,
                            elem_size=ROW_W, queue_num=0)


                    S = pool.tile([P, NB * C], F16, tag="eS", bufs=3)
                    ST = pool.tile([P, NB * C], F16, tag="eST", bufs=3)
                    ocm = pool.tile([P, NB * C], F16, tag="eo", bufs=4)
                    for h in range(2):
                        sl = slice(h * WIN, (h + 1) * WIN)
                        hnb = slice(h * (NB // 2), (h + 1) * (NB // 2))
                        nc.vector.tensor_add(
                            S[:, sl].rearrange("p (n c) -> p n c", c=C),
                            gvv[:, hnb, 0:C], gcv[:, hnb, 0:C])
                        nc.vector.transpose(ST[:, sl], S[:, sl])
                        ps1 = psum.tile([P, WIN], F32, tag="psA")
                        for q in range(2):
                            s2 = slice(h * WIN + q * 512, h * WIN + q * 512 + 512)
                            qs = slice(q * 512, (q + 1) * 512)
                            nc.tensor.matmul(ps1[:, qs], lhsT=W["a_st"][:],
                                             rhs=Xt[:, s2],
                                             start=True, stop=False)
                            nc.tensor.matmul(ps1[:, qs], lhsT=W["ident"][:],
                                             rhs=ST[:, s2],
                                             start=False, stop=True)
                        hh = pool.tile([P, WIN], F16, tag="eh", bufs=4)
                        nc.scalar.activation(hh[:], ps1[:], RELU,
                                             bias=B["be1_t"][:])
                        ps2 = psum.tile([P, WIN], F32, tag="psB")
                        for q in range(2):
                            qs = slice(q * 512, (q + 1) * 512)
                            nc.tensor.matmul(ps2[:, qs], lhsT=W["w2_st"][:],
                                             rhs=hh[:, qs],
                                             start=True, stop=True)
                        if h == 1:
                            nc.vector.tensor_scalar_add(ocm[:, sl], ps2[:],
                                                        B["be2_t"][:])
                        else:
                            nc.scalar.activation(ocm[:, sl], ps2[:], IDENT,
                                                 bias=B["be2_t"][:])
                    nc.sync.dma_start(out[mi], ocm[:])

                # Interleave node chunks with the edge tiles that become
                # ready once their (pv, pc) chunk pair is written. Rank
                # group g = classes with max(cv, cc) == g covers class
                # ranks [g^2, (g+1)^2).
                tiles_per_cls = cls_slots // TILE
                for g in range(NCHUNK):
                    early = g < 0
                    node_chunk(vfull, vpart, pv_tabs[g], g, W["wv1_st"],
                               W["wv2_st"], W["pv_st"], B["bv1_t"],
                               B["bv2_t"], pool_relu1=early)
                    node_chunk(cfull, cpart, pc_tabs[g], g, W["wc1_st"],
                               W["wc2_st"], W["pc_st"], B["bc1_t"],
                               B["bc2_t"], pool_relu1=early)
                    for mi in range(g * g * tiles_per_cls,
                                    (g + 1) * (g + 1) * tiles_per_cls):
                        edge_tile(mi)

    mybir.codegen_inst_isa_subclasses(nc)
    _split_multi_waits(nc)
    return nc


def _kron4(w: np.ndarray) -> np.ndarray:
    return np.kron(np.eye(4, dtype=np.float32), w).astype(np.float16)


def _bias_t(b: np.ndarray) -> np.ndarray:
    return np.tile(np.asarray(b, np.float32), 4)[:, None].astype(np.float32)


def make_weight_inputs(Wv1, bv1, Wv2, bv2, Wc1, bc1, Wc2, bc2,
                       We1, be1, We2, be2):
    We1 = np.asarray(We1, np.float32)
    return {
        "a_st": _kron4(np.asarray(We1[:, :C]).T.astype(np.float32)),
        "ident": np.eye(P, dtype=np.float16),
        "w2_st": _kron4(np.asarray(We2, np.float32).T),
        "wv1_st": _kron4(np.asarray(Wv1, np.float32).T),
        "wv2_st": _kron4(np.asarray(Wv2, np.float32).T),
        "pv_st": _kron4(We1[:, C:2 * C].T),
        "wc1_st": _kron4(np.asarray(Wc1, np.float32).T),
        "wc2_st": _kron4(np.asarray(Wc2, np.float32).T),
        "pc_st": _kron4(We1[:, 2 * C:3 * C].T),
        "bv1_t": _bias_t(bv1),
        "bv2_t": _bias_t(bv2),
        "bc1_t": _bias_t(bc1),
        "bc2_t": _bias_t(bc2),
        "be1_t": _bias_t(be1),
        "be2_t": _bias_t(be2),
    }


def _to_cm(rows: np.ndarray, nb: int) -> np.ndarray:
    """Row-major macro [128*nb, C] (row = p*nb + j) -> channel-major kron4
    [128, nb*C]: out[32a+c, j*32+e] = rows[(32a+e)*nb + j, c]."""
    r5 = rows.reshape(4, C, nb, C)
    return r5.transpose(0, 3, 2, 1).reshape(P, nb * C)


def _stage_nodes(x: np.ndarray):
    """Map nodes n -> table row (n//NODE_CAP)*CHUNK + n%NODE_CAP; build
    channel-major macro inputs (12 full + 4 partial)."""
    n = x.shape[0]
    full = np.zeros((12, P, NB * C), np.float16)
    part = np.zeros((NCHUNK, P, PART_NB * C), np.float16)
    staged = np.zeros((TAB_ROWS, C), np.float32)
    for c in range(NCHUNK):
        lo = c * NODE_CAP
        cnt = max(0, min(NODE_CAP, n - lo))
        if cnt:
            staged[c * CHUNK:c * CHUNK + cnt] = x[lo:lo + cnt]
    for c in range(NCHUNK):
        for m in range(3):
            r0 = c * CHUNK + m * TILE
            full[c * 3 + m] = _to_cm(staged[r0:r0 + TILE], NB)
        r0 = c * CHUNK + 3 * TILE
        part[c] = _to_cm(staged[r0:r0 + P * PART_NB], PART_NB)
    return full, part


_NC_CACHE: dict = {}


def _get_nc(me: int, cls_slots: int) -> bass.Bass:
    key = (me, cls_slots)
    if key not in _NC_CACHE:
        _NC_CACHE[key] = build_nc(me, cls_slots)
    return _NC_CACHE[key]


def kernel(var_f, con_f, combined_edge_f, edge_index_var_con,
           Wv1, bv1, Wv2, bv2, Wc1, bc1, Wc2, bc2, We1, be1, We2, be2,
           _trace=False, _tmpdir=None):
    var_f = np.asarray(var_f, np.float32)
    con_f = np.asarray(con_f, np.float32)
    ef = np.asarray(combined_edge_f, np.float32)
    eidx = np.asarray(edge_index_var_con)
    i0 = eidx[0].astype(np.int64)
    i1 = eidx[1].astype(np.int64)
    E = ef.shape[0]

    chv = i0 // NODE_CAP
    chc = i1 // NODE_CAP
    lov = (i0 - chv * NODE_CAP).astype(np.int16)
    loc = (i1 - chc * NODE_CAP).astype(np.int16)
    cls = (chv * NCHUNK + chc).astype(np.int64)

    per = -(-E // N_CORES)
    # class histogram per core -> slots per class
    max_cnt = 0
    bounds = []
    for k in range(N_CORES):
        lo, hi = k * per, min((k + 1) * per, E)
        bounds.append((lo, hi))
        cnt = np.bincount(cls[lo:hi], minlength=NCLS)
        max_cnt = max(max_cnt, int(cnt.max()))
    cls_slots = max(CLS_SLOTS0, -(-max_cnt // WIN) * WIN)
    me = (NCLS * cls_slots) // TILE
    S = me * TILE

    base = make_weight_inputs(Wv1, bv1, Wv2, bv2, Wc1, bc1, Wc2, bc2,
                              We1, be1, We2, be2)
    base["vfull"], base["vpart"] = _stage_nodes(var_f)
    base["cfull"], base["cpart"] = _stage_nodes(con_f)

    in_maps = []
    orders = []
    for k in range(N_CORES):
        lo, hi = bounds[k]
        ck = np.asarray(CLASS_RANK, np.int64)[cls[lo:hi]]
        order = np.argsort(ck, kind="stable")  # shard-local edge ids
        cnt = np.bincount(ck, minlength=NCLS)
        starts = np.arange(NCLS) * cls_slots
        # slot of sorted edge t: starts[class rank] + rank within class
        slots = np.repeat(starts, cnt) + (
            np.arange(len(order)) - np.repeat(np.cumsum(cnt) - cnt, cnt))
        ef_s = np.zeros((S, C), np.float32)
        ef_s[slots] = ef[lo:hi][order]
        lv_s = np.zeros(S, np.int16)
        lv_s[slots] = lov[lo:hi][order]
        lc_s = np.zeros(S, np.int16)
        lc_s[slots] = loc[lo:hi][order]
        orders.append((order, slots))

        # X: [me, 128, 2048]: X[mi, 32a+c, j*32+e] = ef_s[mi*8192+j*128+32a+e, c]
        e5 = ef_s.reshape(me, NB, 4, C, C)          # [mi, j, a, e, c]
        xcm = np.ascontiguousarray(
            e5.transpose(0, 2, 4, 1, 3)).reshape(me, P, NB * C).astype(
                np.float16)
        # idx wrap: [me, 128, 512] each; slot s at [s%16, s//16], repl. x8;
        # v in cols 0:512, c in cols 512:1024
        iv = np.tile(lv_s.reshape(me, TILE // 16, 16).transpose(0, 2, 1),
                     (1, 8, 1))
        ic = np.tile(lc_s.reshape(me, TILE // 16, 16).transpose(0, 2, 1),
                     (1, 8, 1))
        m = dict(base)
        m["xcm"] = xcm
        m["idx"] = np.ascontiguousarray(np.concatenate([iv, ic], axis=2))
        in_maps.append(m)

    nc = _get_nc(me, cls_slots)
    res = run_bass_kernel_spmd(nc, in_maps, list(range(N_CORES)),
                               trace=_trace, tmpdir=_tmpdir)

    full = np.empty((E, C), np.float32)
    for k in range(N_CORES):
        lo, hi = bounds[k]
        o = np.asarray(res.results[k]["out"]).astype(np.float32)\
            .reshape(me, 4, C, NB, C)
        # out[mi, a, r, j, e] -> slot mi*8192 + j*128 + 32a + e
        slot_out = np.ascontiguousarray(
            o.transpose(0, 3, 1, 4, 2)).reshape(S, C)
        order, slots = orders[k]
        shard = np.empty((hi - lo, C), np.float32)
        shard[order] = slot_out[slots]
        full[lo:hi] = shard
    if _trace:
        return full, res
    return full


# revision 43
# speedup vs baseline: 6.2719x; 1.0259x over previous
"""Trainium2 Bass kernel for nn_EdgeUpdater (GNN message passing edge update).

Computes, for E=2M edges with node tables of 100k rows (C=32):
    v = relu(relu(var_f @ Wv1.T + bv1) @ Wv2.T + bv2)
    c = relu(relu(con_f @ Wc1.T + bc1) @ Wc2.T + bc2)
    x = concat([combined_edge_f, v[i0], c[i1]], axis=1)      # [E, 3C]
    out = relu(x @ We1.T + be1) @ We2.T + be2                # [E, C]

Strategy (8 cores, edge-sharded, node tables replicated per core):
  Algebraic split of We1 = [A | B | Cc] over the concat:
    h  = relu(A@e + Pv[i0] + Pc[i1] + be1),  out = We2@h + be2
    Pv = v @ B.T,  Pc = c @ Cc.T   (be1 applied via the relu ACT bias)
  Per core: compute Pv/Pc tables as f32 rows padded to 64 f32 (256B) in
  DRAM, then stream edge tiles of 8192:
  - The per-edge random gather uses SWDGE dma_gather (InstDMAGatherAnt,
    'mlp' GPSIMD library): 1024 rows per instruction, int16 indices.
    To satisfy the int16 range, nodes are remapped host-side into 4 table
    chunks of 32768 rows, and edges are class-sorted host-side by
    (chunk(i0), chunk(i1)) into 16 fixed-size slot regions per core, so
    every 1024-slot gather window reads one statically-known chunk pair
    (the module stays SPMD-uniform across cores).
  - Edge features arrive host-pre-transposed into the 4x block-diagonal
    (kron(I4, W)) channel-major layout; outputs are stored channel-major
    and un-transposed host-side. Gathered rows are extract-added on DVE
    and 32x32 stream-transposed into the same layout.
"""

import numpy as np

import concourse.bass as bass
import concourse.mybir as mybir
import concourse.tile as tile
from concourse import library_config
from concourse.bass_utils import run_bass_kernel_spmd

C = 32
P = 128
NB = 64                 # j-slots per edge tile
TILE = P * NB           # 8192 edges per tile
WIN = 1024              # idx slots per dma_gather instruction
NWIN = TILE // WIN      # 8 windows per tile
NCHUNK = 4
CHUNK = 32768           # table rows per chunk
TAB_ROWS = NCHUNK * CHUNK
ROW_W = 64              # f32 per table row (256B; payload in cols 0:32)
NODE_CAP = 25088        # real node rows per chunk (3*8192 + 512)
NCLS = NCHUNK * NCHUNK  # 16 (chunk_v, chunk_c) classes
CLS_SLOTS0 = 16384      # default slots per class per core
N_CORES = 8
PART_NB = 4             # j-slots of the partial node macro (512 rows)
# Edge classes (cv, cc) ordered by readiness under the chunk-interleaved
# node schedule pv0,pc0,pv1,pc1,...: class usable once pv[cv] and pc[cc]
# are both written, i.e. after phase max(2*cv+1, 2*cc+2).
CLASS_ORDER = sorted(range(NCLS), key=lambda k: (
    max(2 * (k // NCHUNK) + 1, 2 * (k % NCHUNK) + 2), k))
CLASS_RANK = [CLASS_ORDER.index(k) for k in range(NCLS)]

F32 = mybir.dt.float32
F16 = mybir.dt.float16
I16 = mybir.dt.int16

RELU = mybir.ActivationFunctionType.Relu
IDENT = mybir.ActivationFunctionType.Identity


def _split_multi_waits(nc: bass.Bass, max_waits: int = 1):
    """The walrus in this container rejects instructions carrying more than
    one sync wait ("Too many sync wait commands", CoreV2/V3 setupSyncWait).
    Hoist extra waits onto single-wait NOPs on the same engine, inserted
    immediately before the instruction (same per-engine program order, so
    semantics are unchanged)."""
    for fn in nc.m.functions:
        for bb in fn.blocks:
            insts = list(bb.instructions)
            out = []
            for ins in insts:
                si = ins.sync_info
                if (si is not None and si.on_wait
                        and len(si.on_wait) > max_waits
                        and ins.engine is not None):
                    waits = list(si.on_wait)
                    eng = nc.engines[ins.engine]
                    for w in waits[:-max_waits]:
                        nop = eng.nop()
                        cur = nc.cur_bb.bb
                        assert cur.instructions[-1] is nop.ins
                        cur.instructions.pop()
                        nop.ins.sync_info = mybir.SyncInfo(
                            on_wait=[w], on_update=[])
                        out.append(nop.ins)
                    si.on_wait = waits[-max_waits:]
                out.append(ins)
            bb.instructions.clear()
            for ins in out:
                bb.instructions.append(ins)


def build_nc(me: int, cls_slots: int) -> bass.Bass:
    """Per-core module. me = edge tiles; cls_slots = slots per class
    (multiple of 1024; me*TILE == NCLS*cls_slots)."""
    assert me * TILE == NCLS * cls_slots
    wins_per_cls = cls_slots // WIN

    nc = bass.Bass()

    xcm = nc.declare_dram_parameter("xcm", [me, P, NB * C], F16, isOutput=False)
    idx = nc.declare_dram_parameter("idx", [me, P, TILE // 8], I16,
                                    isOutput=False)
    vfull = nc.declare_dram_parameter("vfull", [12, P, NB * C], F16,
                                      isOutput=False)
    vpart = nc.declare_dram_parameter("vpart", [NCHUNK, P, PART_NB * C], F16,
                                      isOutput=False)
    cfull = nc.declare_dram_parameter("cfull", [12, P, NB * C], F16,
                                      isOutput=False)
    cpart = nc.declare_dram_parameter("cpart", [NCHUNK, P, PART_NB * C], F16,
                                      isOutput=False)

    wnames = ["a_st", "ident", "w2_st", "wv1_st", "wv2_st", "pv_st",
              "wc1_st", "wc2_st", "pc_st"]
    wpack = nc.declare_dram_parameter("wpack", [P, 9 * P], F16,
                                      isOutput=False)
    bnames = ["bv1_t", "bv2_t", "bc1_t", "bc2_t", "be1_t", "be2_t"]
    bpack = nc.declare_dram_parameter("bpack", [P, 6], F32, isOutput=False)

    out = nc.declare_dram_parameter("out", [me, P, NB * C], F16, isOutput=True)

    pv_tabs = [nc.dram_tensor(f"pv_tab{c}", [CHUNK, ROW_W], F32)
               for c in range(NCHUNK)]
    pc_tabs = [nc.dram_tensor(f"pc_tab{c}", [CHUNK, ROW_W], F32)
               for c in range(NCHUNK)]

    with tile.TileContext(nc) as tc:
        nc.gpsimd.load_library(library_config.mlp)
        win_reg = nc.gpsimd.to_reg(WIN)
        # Preload an activation-function table containing both Relu and
        # Identity so no activation pays an implicit ACT_TABLE_LOAD.
        nc.scalar.add_instruction(mybir.InstLoadActFuncSet(
            name=nc.get_next_instruction_name(), ins=[], outs=[],
            act_func_set_id=0))
        with tc.tile_pool(name="const", bufs=1) as cpool:
            wpack_t = cpool.tile([P, len(wnames) * P], F16, tag="wpack")
            nc.sync.dma_start(wpack_t[:], wpack[:])
            W = {n: wpack_t[:, i * P:(i + 1) * P]
                 for i, n in enumerate(wnames)}
            bpack_t = cpool.tile([P, len(bnames)], F32, tag="bpack")
            nc.scalar.dma_start(bpack_t[:], bpack[:])
            B = {n: bpack_t[:, i:i + 1] for i, n in enumerate(bnames)}
            with tc.tile_pool(name="sb", bufs=2) as pool, \
                 tc.tile_pool(name="psum", bufs=2, space="PSUM") as psum:

                def node_macro(src, src_i, tab, r0, nb, w1, w2, w3, b1, b2,
                               pool_relu1=False):
                    """MLP a macro of 128*nb node rows; write f32 rows
                    [r0 + p*64 + j] cols 0:32 of tab."""
                    fr = nb * C
                    X = pool.tile([P, NB * C], F16, tag="nX")
                    nc.sync.dma_start(X[:, :fr], src[src_i])
                    rm = pool.tile([P, NB * C], F32, tag="nrm")
                    nh = max(1, fr // WIN)
                    for h in range(nh):
                        w = min(WIN, fr)
                        sl = slice(h * WIN, h * WIN + w)
                        pA = psum.tile([P, WIN], F32, tag="psA")
                        for q0 in range(0, w, 512):
                            qw = min(512, w - q0)
                            nc.tensor.matmul(pA[:, q0:q0 + qw],
                                             lhsT=w1,
                                             rhs=X[:, h * WIN + q0:
                                                  h * WIN + q0 + qw],
                                             start=True, stop=True)
                        l1 = pool.tile([P, WIN], F16, tag="nl1")
                        if pool_relu1:
                            nc.gpsimd.tensor_scalar(
                                l1[:, :w], pA[:, :w], b1[:], 0.0,
                                mybir.AluOpType.add, mybir.AluOpType.max)
                        else:
                            nc.scalar.activation(l1[:, :w], pA[:, :w], RELU,
                                                 bias=b1)
                        pB = psum.tile([P, WIN], F32, tag="psB")
                        for q0 in range(0, w, 512):
                            qw = min(512, w - q0)
                            nc.tensor.matmul(pB[:, q0:q0 + qw],
                                             lhsT=w2,
                                             rhs=l1[:, q0:q0 + qw],
                                             start=True, stop=True)
                        l2 = pool.tile([P, WIN], F16, tag="nl2")
                        if pool_relu1:
                            nc.gpsimd.tensor_scalar(
                                l2[:, :w], pB[:, :w], b2[:], 0.0,
                                mybir.AluOpType.add, mybir.AluOpType.max)
                        else:
                            nc.scalar.activation(l2[:, :w], pB[:, :w], RELU,
                                                 bias=b2)
                        pC = psum.tile([P, WIN], F32, tag="psA")
                        for q0 in range(0, w, 512):
                            qw = min(512, w - q0)
                            nc.tensor.matmul(pC[:, q0:q0 + qw],
                                             lhsT=w3,
                                             rhs=l2[:, q0:q0 + qw],
                                             start=True, stop=True)
                        nc.vector.transpose(rm[:, sl], pC[:, :w])
                    # rm[p, j*C + c] = table value for row r0 + p*nb + j
                    rows = tab[r0:r0 + P * nb]
                    tv = rows.rearrange("(p j) w -> p j w", p=P)
                    nc.sync.dma_start(tv[:, :, 0:C],
                                  rm[:, :fr].rearrange("p (j c) -> p j c", c=C))

                def node_chunk(src_full, src_part, tab, c, w1, w2, w3,
                               b1, b2, pool_relu1=False):
                    for m in range(3):
                        node_macro(src_full, c * 3 + m, tab, m * TILE, NB,
                                   w1, w2, w3, b1, b2, pool_relu1)
                    node_macro(src_part, c, tab, 3 * TILE, PART_NB,
                               w1, w2, w3, b1, b2, pool_relu1)

                def edge_tile(mi):
                    Xt = pool.tile([P, NB * C], F16, tag="eX", bufs=3)
                    nc.sync.dma_start(Xt[:], xcm[mi])
                    tix = pool.tile([P, TILE // 8], I16, tag="eix", bufs=4)
                    nc.sync.dma_start(tix[:], idx[mi])

                    Gv = pool.tile([P, NB * ROW_W], F32, tag="eGv", bufs=3)
                    Gc = pool.tile([P, NB * ROW_W], F32, tag="eGc", bufs=3)
                    gvv = Gv[:].rearrange("p (n e) -> p n e", e=ROW_W)
                    gcv = Gc[:].rearrange("p (n e) -> p n e", e=ROW_W)
                    jw = WIN // P  # 8 j-slots per window
                    for w in range(NWIN):
                        k = CLASS_ORDER[(mi * NWIN + w) // wins_per_cls]
                        nc.gpsimd.dma_gather(
                            out_ap=gvv[:, w * jw:(w + 1) * jw],
                            in_ap=pv_tabs[k // NCHUNK][:],
                            idxs_ap=tix[:, w * (WIN // 16):(w + 1) * (WIN // 16)],
                            num_idxs=WIN, num_idxs_reg=win_reg,
                            elem_size=ROW_W, queue_num=0)
                        nc.gpsimd.dma_gather(
                            out_ap=gcv[:, w * jw:(w + 1) * jw],
                            in_ap=pc_tabs[k % NCHUNK][:],
                            idxs_ap=tix[:, TILE // 16 + w * (WIN // 16):
                                         TILE // 16 + (w + 1) * (WIN // 16)],
                            num_idxs=WIN, num_idxs_reg=win_reg,
                            elem_size=ROW_W, queue_num=0)


                    S = pool.tile([P, NB * C], F16, tag="eS", bufs=3)
                    ST = pool.tile([P, NB * C], F16, tag="eST", bufs=3)
                    ocm = pool.tile([P, NB * C], F16, tag="eo", bufs=4)
                    for h in range(2):
                        sl = slice(h * WIN, (h + 1) * WIN)
                        hnb = slice(h * (NB // 2), (h + 1) * (NB // 2))
                        nc.vector.tensor_add(
                            S[:, sl].rearrange("p (n c) -> p n c", c=C),
                            gvv[:, hnb, 0:C], gcv[:, hnb, 0:C])
                        nc.vector.transpose(ST[:, sl], S[:, sl])
                        ps1 = psum.tile([P, WIN], F32, tag="psA")
                        for q in range(2):
                            s2 = slice(h * WIN + q * 512, h * WIN + q * 512 + 512)
                            qs = slice(q * 512, (q + 1) * 512)
                            nc.tensor.matmul(ps1[:, qs], lhsT=W["a_st"],
                                             rhs=Xt[:, s2],
                                             start=True, stop=False)
                            nc.tensor.matmul(ps1[:, qs], lhsT=W["ident"],
                                             rhs=ST[:, s2],
                                             start=False, stop=True)
                        hh = pool.tile([P, WIN], F16, tag="eh", bufs=4)
                        nc.scalar.activation(hh[:], ps1[:], RELU,
                                             bias=B["be1_t"])
                        ps2 = psum.tile([P, WIN], F32, tag="psB")
                        for q in range(2):
                            qs = slice(q * 512, (q + 1) * 512)
                            nc.tensor.matmul(ps2[:, qs], lhsT=W["w2_st"],
                                             rhs=hh[:, qs],
                                             start=True, stop=True)
                        if h == 1:
                            nc.vector.tensor_scalar_add(ocm[:, sl], ps2[:],
                                                        B["be2_t"][:])
                        else:
                            nc.scalar.activation(ocm[:, sl], ps2[:], IDENT,
                                                 bias=B["be2_t"])
                    nc.sync.dma_start(out[mi], ocm[:])

                # Interleave node chunks with the edge tiles that become
                # ready once their (pv, pc) chunk pair is written. Rank
                # group g = classes with max(cv, cc) == g covers class
                # ranks [g^2, (g+1)^2).
                tiles_per_cls = cls_slots // TILE
                for g in range(NCHUNK):
                    early = g < 0
                    node_chunk(vfull, vpart, pv_tabs[g], g, W["wv1_st"],
                               W["wv2_st"], W["pv_st"], B["bv1_t"],
                               B["bv2_t"], pool_relu1=early)
                    node_chunk(cfull, cpart, pc_tabs[g], g, W["wc1_st"],
                               W["wc2_st"], W["pc_st"], B["bc1_t"],
                               B["bc2_t"], pool_relu1=early)
                    for mi in range(g * g * tiles_per_cls,
                                    (g + 1) * (g + 1) * tiles_per_cls):
                        edge_tile(mi)

    mybir.codegen_inst_isa_subclasses(nc)
    _split_multi_waits(nc)
    return nc


def _kron4(w: np.ndarray) -> np.ndarray:
    return np.kron(np.eye(4, dtype=np.float32), w).astype(np.float16)


def _bias_t(b: np.ndarray) -> np.ndarray:
    return np.tile(np.asarray(b, np.float32), 4)[:, None].astype(np.float32)


def make_weight_inputs(Wv1, bv1, Wv2, bv2, Wc1, bc1, Wc2, bc2,
                       We1, be1, We2, be2):
    We1 = np.asarray(We1, np.float32)
    d = {
        "a_st": _kron4(np.asarray(We1[:, :C]).T.astype(np.float32)),
        "ident": np.eye(P, dtype=np.float16),
        "w2_st": _kron4(np.asarray(We2, np.float32).T),
        "wv1_st": _kron4(np.asarray(Wv1, np.float32).T),
        "wv2_st": _kron4(np.asarray(Wv2, np.float32).T),
        "pv_st": _kron4(We1[:, C:2 * C].T),
        "wc1_st": _kron4(np.asarray(Wc1, np.float32).T),
        "wc2_st": _kron4(np.asarray(Wc2, np.float32).T),
        "pc_st": _kron4(We1[:, 2 * C:3 * C].T),
        "bv1_t": _bias_t(bv1),
        "bv2_t": _bias_t(bv2),
        "bc1_t": _bias_t(bc1),
        "bc2_t": _bias_t(bc2),
        "be1_t": _bias_t(be1),
        "be2_t": _bias_t(be2),
    }
    wnames = ["a_st", "ident", "w2_st", "wv1_st", "wv2_st", "pv_st",
              "wc1_st", "wc2_st", "pc_st"]
    bnames = ["bv1_t", "bv2_t", "bc1_t", "bc2_t", "be1_t", "be2_t"]
    return {
        "wpack": np.concatenate([d[n] for n in wnames], axis=1),
        "bpack": np.concatenate([d[n] for n in bnames], axis=1),
    }


def _to_cm(rows: np.ndarray, nb: int) -> np.ndarray:
    """Row-major macro [128*nb, C] (row = p*nb + j) -> channel-major kron4
    [128, nb*C]: out[32a+c, j*32+e] = rows[(32a+e)*nb + j, c]."""
    r5 = rows.reshape(4, C, nb, C)
    return r5.transpose(0, 3, 2, 1).reshape(P, nb * C)


def _stage_nodes(x: np.ndarray):
    """Map nodes n -> table row (n//NODE_CAP)*CHUNK + n%NODE_CAP; build
    channel-major macro inputs (12 full + 4 partial)."""
    n = x.shape[0]
    full = np.zeros((12, P, NB * C), np.float16)
    part = np.zeros((NCHUNK, P, PART_NB * C), np.float16)
    staged = np.zeros((TAB_ROWS, C), np.float32)
    for c in range(NCHUNK):
        lo = c * NODE_CAP
        cnt = max(0, min(NODE_CAP, n - lo))
        if cnt:
            staged[c * CHUNK:c * CHUNK + cnt] = x[lo:lo + cnt]
    for c in range(NCHUNK):
        for m in range(3):
            r0 = c * CHUNK + m * TILE
            full[c * 3 + m] = _to_cm(staged[r0:r0 + TILE], NB)
        r0 = c * CHUNK + 3 * TILE
        part[c] = _to_cm(staged[r0:r0 + P * PART_NB], PART_NB)
    return full, part


_NC_CACHE: dict = {}


def _get_nc(me: int, cls_slots: int) -> bass.Bass:
    key = (me, cls_slots)
    if key not in _NC_CACHE:
        _NC_CACHE[key] = build_nc(me, cls_slots)
    return _NC_CACHE[key]


def kernel(var_f, con_f, combined_edge_f, edge_index_var_con,
           Wv1, bv1, Wv2, bv2, Wc1, bc1, Wc2, bc2, We1, be1, We2, be2,
           _trace=False, _tmpdir=None):
    var_f = np.asarray(var_f, np.float32)
    con_f = np.asarray(con_f, np.float32)
    ef = np.asarray(combined_edge_f, np.float32)
    eidx = np.asarray(edge_index_var_con)
    i0 = eidx[0].astype(np.int64)
    i1 = eidx[1].astype(np.int64)
    E = ef.shape[0]

    chv = i0 // NODE_CAP
    chc = i1 // NODE_CAP
    lov = (i0 - chv * NODE_CAP).astype(np.int16)
    loc = (i1 - chc * NODE_CAP).astype(np.int16)
    cls = (chv * NCHUNK + chc).astype(np.int64)

    per = -(-E // N_CORES)
    # class histogram per core -> slots per class
    max_cnt = 0
    bounds = []
    for k in range(N_CORES):
        lo, hi = k * per, min((k + 1) * per, E)
        bounds.append((lo, hi))
        cnt = np.bincount(cls[lo:hi], minlength=NCLS)
        max_cnt = max(max_cnt, int(cnt.max()))
    cls_slots = max(CLS_SLOTS0, -(-max_cnt // WIN) * WIN)
    me = (NCLS * cls_slots) // TILE
    S = me * TILE

    base = make_weight_inputs(Wv1, bv1, Wv2, bv2, Wc1, bc1, Wc2, bc2,
                              We1, be1, We2, be2)
    base["vfull"], base["vpart"] = _stage_nodes(var_f)
    base["cfull"], base["cpart"] = _stage_nodes(con_f)

    in_maps = []
    orders = []
    for k in range(N_CORES):
        lo, hi = bounds[k]
        ck = np.asarray(CLASS_RANK, np.int64)[cls[lo:hi]]
        order = np.argsort(ck, kind="stable")  # shard-local edge ids
        cnt = np.bincount(ck, minlength=NCLS)
        starts = np.arange(NCLS) * cls_slots
        # slot of sorted edge t: starts[class rank] + rank within class
        slots = np.repeat(starts, cnt) + (
            np.arange(len(order)) - np.repeat(np.cumsum(cnt) - cnt, cnt))
        ef_s = np.zeros((S, C), np.float32)
        ef_s[slots] = ef[lo:hi][order]
        lv_s = np.zeros(S, np.int16)
        lv_s[slots] = lov[lo:hi][order]
        lc_s = np.zeros(S, np.int16)
        lc_s[slots] = loc[lo:hi][order]
        orders.append((order, slots))

        # X: [me, 128, 2048]: X[mi, 32a+c, j*32+e] = ef_s[mi*8192+j*128+32a+e, c]
        e5 = ef_s.reshape(me, NB, 4, C, C)          # [mi, j, a, e, c]
        xcm = np.ascontiguousarray(
            e5.transpose(0, 2, 4, 1, 3)).reshape(me, P, NB * C).astype(
                np.float16)
        # idx wrap: [me, 128, 512] each; slot s at [s%16, s//16], repl. x8;
        # v in cols 0:512, c in cols 512:1024
        iv = np.tile(lv_s.reshape(me, TILE // 16, 16).transpose(0, 2, 1),
                     (1, 8, 1))
        ic = np.tile(lc_s.reshape(me, TILE // 16, 16).transpose(0, 2, 1),
                     (1, 8, 1))
        m = dict(base)
        m["xcm"] = xcm
        m["idx"] = np.ascontiguousarray(np.concatenate([iv, ic], axis=2))
        in_maps.append(m)

    nc = _get_nc(me, cls_slots)
    res = run_bass_kernel_spmd(nc, in_maps, list(range(N_CORES)),
                               trace=_trace, tmpdir=_tmpdir)

    full = np.empty((E, C), np.float32)
    for k in range(N_CORES):
        lo, hi = bounds[k]
        o = np.asarray(res.results[k]["out"]).astype(np.float32)\
            .reshape(me, 4, C, NB, C)
        # out[mi, a, r, j, e] -> slot mi*8192 + j*128 + 32a + e
        slot_out = np.ascontiguousarray(
            o.transpose(0, 3, 1, 4, 2)).reshape(S, C)
        order, slots = orders[k]
        shard = np.empty((hi - lo, C), np.float32)
        shard[order] = slot_out[slots]
        full[lo:hi] = shard
    if _trace:
        return full, res
    return full
